# revision 6
# baseline (speedup 1.0000x reference)
"""Trainium2 Bass kernel for Encoder_Baseline (GRU_with_GCC over stream 1).

Strategy: data-parallel over batch (B=128 -> 16 per core, 8 cores, SPMD).
Phase 1 (per core): A[bt, 0:1024] = feat1 @ W_ih.T[:, 0:1024] + feat0 @ W_fh.T
                                    + (b_ih + b_fh + b_hh)[0:1024]
                    A[bt, 1024:1536] = feat1 @ W_ih.T[:, 1024:] + b_ih[1024:]
   computed with fp32r matmuls, biases injected via K=1 ones-matmul,
   A kept resident in SBUF (bf16).
Phase 2: 64-step GRU recurrence, batch-major [16, *] tiles; h.T kept as the
   matmul stationary operand; b_hh[1024:] injected into PSUM each step;
   h transposed back via 4 PE transposes per step.
"""

import sys

for _p in ("/opt/trn_rl_repo",):
    if _p not in sys.path:
        sys.path.insert(0, _p)

from contextlib import ExitStack

import numpy as np

import concourse.bass as bass
import concourse.mybir as mybir
import concourse.tile as tile
from concourse import bacc
from concourse.bass_utils import run_bass_kernel_spmd
from concourse.masks import make_identity

F32 = mybir.dt.float32
F32R = mybir.dt.float32r
BF16 = mybir.dt.bfloat16
AF = mybir.ActivationFunctionType
ALU = mybir.AluOpType

B, T, H, IN, FS = 128, 64, 512, 2048, 1536
NC = 8
BL = B // NC          # 16 batch rows per core
BT = BL * T           # 1024 rows per core
H3 = 3 * H            # 1536
K_IN = IN // 128      # 16
K_FS = FS // 128      # 12
K_H = H // 128        # 4

_cache = {}


def _build_program():
    nc = bacc.Bacc("TRN2", target_bir_lowering=False, debug=False)

    f1T = nc.dram_tensor("f1T", [IN, BT], F32R, kind="ExternalInput").ap()
    f0T = nc.dram_tensor("f0T", [FS, BT], F32R, kind="ExternalInput").ap()
    wihT = nc.dram_tensor("wihT", [IN, H3], F32R, kind="ExternalInput").ap()
    wfhT = nc.dram_tensor("wfhT", [FS, 2 * H], F32R, kind="ExternalInput").ap()
    whhT = nc.dram_tensor("whhT", [H, H3], F32R, kind="ExternalInput").ap()
    bri = nc.dram_tensor("bri", [1, 2 * H], F32R, kind="ExternalInput").ap()
    bnin = nc.dram_tensor("bnin", [1, H], F32R, kind="ExternalInput").ap()
    bnh = nc.dram_tensor("bnh", [1, H], F32R, kind="ExternalInput").ap()
    ones_d = nc.dram_tensor("ones_d", [1, 128], F32R, kind="ExternalInput").ap()
    zeros_d = nc.dram_tensor("zeros_d", [128, 64], F32R, kind="ExternalInput").ap()
    eo = nc.dram_tensor("eo", [BL, T, H], F32, kind="ExternalOutput").ap()

    with tile.TileContext(nc) as tc, ExitStack() as ctx:
        const = ctx.enter_context(tc.tile_pool(name="const", bufs=1))
        ident = const.tile([BL, BL], F32, tag="ident")
        make_identity(nc, ident[:])
        ones = const.tile([1, 128], F32R, tag="ones")
        nc.sync.dma_start(ones[:], ones_d[:])
        bri_s = const.tile([1, 2 * H], F32R, tag="bri")
        nc.sync.dma_start(bri_s[:], bri[:])
        bnin_s = const.tile([1, H], F32R, tag="bnin")
        nc.sync.dma_start(bnin_s[:], bnin[:])
        bnh_s = const.tile([1, H], F32R, tag="bnh")
        nc.sync.dma_start(bnh_s[:], bnh[:])

        whh_pool = ctx.enter_context(tc.tile_pool(name="whh_pool", bufs=1))
        whh_t = []
        for k in range(K_H):
            tt = whh_pool.tile([128, H3], F32R, tag=f"whh_{k}")
            nc.sync.dma_start(tt[:], whhT[128 * k : 128 * (k + 1), :])
            whh_t.append(tt)

        a_pool = ctx.enter_context(tc.tile_pool(name="a_pool", bufs=1))
        a_t = [a_pool.tile([128, H3], BF16, tag=f"a_{bt}", name=f"a_{bt}") for bt in range(8)]

        # ---- Phase 1: A = feat1 @ wihT + feat0 @ wfhT + biases ----
        with tc.tile_pool(name="feats", bufs=1) as feats, tc.tile_pool(
            name="wstream", bufs=4
        ) as wstream, tc.tile_pool(name="pre_psum", bufs=1, space="PSUM") as pre_psum:
            f1_t = []
            for k in range(K_IN):
                tt = feats.tile([128, BT], F32R, tag=f"f1_{k}")
                nc.sync.dma_start(tt[:], f1T[128 * k : 128 * (k + 1), :])
                f1_t.append(tt)
            f0_t = []
            for k in range(K_FS):
                tt = feats.tile([128, BT], F32R, tag=f"f0_{k}")
                nc.sync.dma_start(tt[:], f0T[128 * k : 128 * (k + 1), :])
                f0_t.append(tt)
            for p in range(3):  # 512-wide chunks of 3H
                lo = 512 * p
                ps = [pre_psum.tile([128, 512], F32, tag=f"ps{bt}", name=f"ps{bt}") for bt in range(8)]
                # bias inject (K=1 ones matmul broadcasts the bias row)
                bias_src = bri_s[:, lo : lo + 512] if p < 2 else bnin_s[:]
                for bt in range(8):
                    nc.tensor.matmul(
                        ps[bt][:],
                        ones[:, :128],
                        bias_src,
                        start=True,
                        stop=False,
                    )
                for k in range(K_IN):
                    wt = wstream.tile([128, 512], F32R, tag="w")
                    nc.sync.dma_start(wt[:], wihT[128 * k : 128 * (k + 1), lo : lo + 512])
                    last = (p == 2) and (k == K_IN - 1)
                    for bt in range(8):
                        nc.tensor.matmul(
                            ps[bt][:],
                            f1_t[k][:, 128 * bt : 128 * (bt + 1)],
                            wt[:],
                            start=False,
                            stop=last,
                        )
                if p < 2:
                    for k in range(K_FS):
                        wt = wstream.tile([128, 512], F32R, tag="w")
                        nc.sync.dma_start(
                            wt[:], wfhT[128 * k : 128 * (k + 1), lo : lo + 512]
                        )
                        for bt in range(8):
                            nc.tensor.matmul(
                                ps[bt][:],
                                f0_t[k][:, 128 * bt : 128 * (bt + 1)],
                                wt[:],
                                start=False,
                                stop=(k == K_FS - 1),
                            )
                for bt in range(8):
                    nc.vector.tensor_copy(a_t[bt][:, lo : lo + 512], ps[bt][:])

        # ---- Phase 2: GRU recurrence ----
        h_pool = ctx.enter_context(tc.tile_pool(name="h_pool", bufs=2))
        hT_pool = ctx.enter_context(tc.tile_pool(name="hT_pool", bufs=2))
        g_pool = ctx.enter_context(tc.tile_pool(name="g_pool", bufs=3))
        rec_psum = ctx.enter_context(tc.tile_pool(name="rec_psum", bufs=2, space="PSUM"))
        t_psum = ctx.enter_context(tc.tile_pool(name="t_psum", bufs=2, space="PSUM"))

        h = h_pool.tile([BL, H], F32, tag="h")
        nc.vector.memset(h[:], 0.0)
        hT = hT_pool.tile([128, K_H * BL], F32R, tag="hT")
        nc.sync.dma_start(hT[:], zeros_d[:])

        at_pool = ctx.enter_context(tc.tile_pool(name="at_pool", bufs=4))

        for t in range(T):
            at = at_pool.tile([BL, H3], BF16, tag="at")
            nc.sync.dma_start(
                at[:], a_t[t // 8][BL * (t % 8) : BL * (t % 8) + BL, :]
            )
            ps = rec_psum.tile([BL, H3], F32, tag="ps")
            # inject b_hh[n] into the n-gate chunk
            nc.tensor.matmul(
                ps[:, 1024:1536],
                ones[:, :BL],
                bnh_s[:],
                start=True,
                stop=False,
            )
            for n in range(3):
                for k in range(K_H):
                    nc.tensor.matmul(
                        ps[:, 512 * n : 512 * (n + 1)],
                        hT[:, BL * k : BL * (k + 1)],
                        whh_t[k][:, 512 * n : 512 * (n + 1)],
                        start=(k == 0 and n < 2),
                        stop=(k == K_H - 1),
                    )
            pre = g_pool.tile([BL, 2 * H], F32, tag="pre")
            nc.vector.tensor_add(pre[:], ps[:, : 2 * H], at[:, : 2 * H])
            ri = g_pool.tile([BL, 2 * H], F32, tag="ri")
            nc.scalar.activation(ri[:], pre[:], AF.Sigmoid)
            tmp = g_pool.tile([BL, H], F32, tag="tmp")
            nc.vector.scalar_tensor_tensor(
                tmp[:], ps[:, 2 * H :], 0.0, ri[:, :H], ALU.bypass, ALU.mult
            )
            tmp2 = g_pool.tile([BL, H], F32, tag="tmp2")
            nc.vector.tensor_add(tmp2[:], tmp[:], at[:, 2 * H :])
            nt = g_pool.tile([BL, H], F32, tag="nt")
            nc.scalar.activation(nt[:], tmp2[:], AF.Tanh)
            d = g_pool.tile([BL, H], F32, tag="d")
            nc.vector.tensor_sub(d[:], h[:], nt[:])
            e = g_pool.tile([BL, H], F32, tag="e")
            nc.vector.tensor_mul(e[:], ri[:, H:], d[:])
            h_new = h_pool.tile([BL, H], F32, tag="h")
            nc.vector.tensor_add(h_new[:], nt[:], e[:])
            nc.sync.dma_start(eo[:, t, :], h_new[:])
            if t < T - 1:
                pst = t_psum.tile([128, K_H * BL], F32, tag="pst")
                for j in range(K_H):
                    nc.tensor.transpose(
                        pst[:, BL * j : BL * (j + 1)],
                        h_new[:, 128 * j : 128 * (j + 1)],
                        ident[:],
                    )
                hT_new = hT_pool.tile([128, K_H * BL], F32R, tag="hT")
                nc.vector.tensor_copy(hT_new[:], pst[:])
                h, hT = h_new, hT_new

    nc.compile()
    return nc


def _prep_inputs(feat0, feat1, W_ih, b_ih, W_hh, b_hh, W_fh, b_fh):
    wihT = np.ascontiguousarray(W_ih.T)
    wfhT = np.ascontiguousarray(W_fh.T)
    whhT = np.ascontiguousarray(W_hh.T)
    bri = np.ascontiguousarray(
        (b_ih[: 2 * H] + b_fh + b_hh[: 2 * H]).reshape(1, 2 * H)
    )
    bnin = np.ascontiguousarray(b_ih[2 * H :].reshape(1, H))
    bnh = np.ascontiguousarray(b_hh[2 * H :].reshape(1, H))
    in_maps = []
    for c in range(NC):
        sl = slice(BL * c, BL * (c + 1))
        # column order bt' = t*BL + b so step t reads 16 contiguous partitions
        f1c = np.ascontiguousarray(
            feat1[sl].transpose(2, 1, 0).reshape(IN, BT)
        )  # [IN, (t, b)]
        f0c = np.ascontiguousarray(feat0[sl].transpose(2, 1, 0).reshape(FS, BT))
        in_maps.append(
            {
                "f1T": f1c,
                "f0T": f0c,
                "wihT": wihT,
                "wfhT": wfhT,
                "whhT": whhT,
                "bri": bri,
                "bnin": bnin,
                "bnh": bnh,
                "ones_d": np.ones((1, 128), np.float32),
                "zeros_d": np.zeros((128, 64), np.float32),
            }
        )
    return in_maps


def kernel(feat0, feat1, W_ih, b_ih, W_hh, b_hh, W_fh, b_fh):
    feat0 = np.asarray(feat0, np.float32)
    feat1 = np.asarray(feat1, np.float32)
    W_ih = np.asarray(W_ih, np.float32)
    b_ih = np.asarray(b_ih, np.float32)
    W_hh = np.asarray(W_hh, np.float32)
    b_hh = np.asarray(b_hh, np.float32)
    W_fh = np.asarray(W_fh, np.float32)
    b_fh = np.asarray(b_fh, np.float32)

    if "nc" not in _cache:
        _cache["nc"] = _build_program()
    nc = _cache["nc"]

    in_maps = _prep_inputs(feat0, feat1, W_ih, b_ih, W_hh, b_hh, W_fh, b_fh)
    res = run_bass_kernel_spmd(nc, in_maps, core_ids=list(range(NC)))
    eo = np.empty((B, T, H), np.float32)
    for c in range(NC):
        eo[BL * c : BL * (c + 1)] = res.results[c]["eo"]
    hT = np.ascontiguousarray(eo[:, -1, :])
    return eo, hT


# revision 24
# speedup vs baseline: 143.7039x; 143.7039x over previous
"""Trainium2 Bass kernel for Encoder_Baseline (GRU_with_GCC over stream 1).

Strategy: data-parallel over batch (B=128 -> 16 per core, 8 cores, SPMD).
Phase 1 (per core): A[bt, 0:1024] = feat1 @ W_ih.T[:, 0:1024] + feat0 @ W_fh.T
                                    + (b_ih + b_fh + b_hh)[0:1024]
                    A[bt, 1024:1536] = feat1 @ W_ih.T[:, 1024:] + b_ih[1024:]
   fp32r matmuls, biases injected via K=1 ones-matmul, A resident in SBUF
   (bf16).  The n-chunk pass is emitted tile-by-tile interleaved with the
   recurrence so its matmuls fill PE idle gaps during the gate chains.
Phase 2: 64-step GRU recurrence, batch-major [16, *] tiles; h.T kept as the
   matmul stationary operand (f32r); per-step A_r/A_i and b_hh[n] injected
   into PSUM by small matmuls (off the critical path); gates in bf16;
   h transposed back via 4 PE transposes per step.
"""

import sys

for _p in ("/opt/trn_rl_repo",):
    if _p not in sys.path:
        sys.path.insert(0, _p)

from contextlib import ExitStack

import numpy as np

import concourse.bass as bass
import concourse.mybir as mybir
import concourse.tile as tile
from concourse import bacc
from concourse.bass_utils import run_bass_kernel_spmd
from concourse.masks import make_identity

F32 = mybir.dt.float32
F32R = mybir.dt.float32r
BF16 = mybir.dt.bfloat16
AF = mybir.ActivationFunctionType
ALU = mybir.AluOpType

B, T, H, IN, FS = 128, 64, 512, 2048, 1536
NC = 8
BL = B // NC          # 16 batch rows per core
BT = BL * T           # 1024 rows per core
H3 = 3 * H            # 1536
K_IN = IN // 128      # 16
K_FS = FS // 128      # 12
K_H = H // 128        # 4

_cache = {}


def _build_program():
    nc = bacc.Bacc("TRN2", target_bir_lowering=False, debug=False)

    f1T = nc.dram_tensor("f1T", [IN, BT], F32R, kind="ExternalInput").ap()
    f0T = nc.dram_tensor("f0T", [FS, BT], F32R, kind="ExternalInput").ap()
    wihT = nc.dram_tensor("wihT", [IN, H3], F32R, kind="ExternalInput").ap()
    wfhT = nc.dram_tensor("wfhT", [FS, 2 * H], F32R, kind="ExternalInput").ap()
    whhT = nc.dram_tensor("whhT", [H, H3], F32R, kind="ExternalInput").ap()
    bri = nc.dram_tensor("bri", [1, 2 * H], F32R, kind="ExternalInput").ap()
    bnin = nc.dram_tensor("bnin", [1, H], F32R, kind="ExternalInput").ap()
    bnh = nc.dram_tensor("bnh", [1, H], F32R, kind="ExternalInput").ap()
    ones_d = nc.dram_tensor("ones_d", [1, 128], F32R, kind="ExternalInput").ap()
    zeros_d = nc.dram_tensor("zeros_d", [128, 64], F32R, kind="ExternalInput").ap()
    identb_d = nc.dram_tensor("identb_d", [BL, BL], BF16, kind="ExternalInput").ap()
    eo = nc.dram_tensor("eo", [BL, T, H], F32, kind="ExternalOutput").ap()

    with tile.TileContext(nc) as tc, ExitStack() as ctx:
        const = ctx.enter_context(tc.tile_pool(name="const", bufs=1))
        ident = const.tile([BL, BL], F32, tag="ident")
        make_identity(nc, ident[:])
        identb = const.tile([BL, BL], BF16, tag="identb")
        nc.sync.dma_start(identb[:], identb_d[:])
        ones = const.tile([1, 128], F32R, tag="ones")
        nc.sync.dma_start(ones[:], ones_d[:])
        bri_s = const.tile([1, 2 * H], F32R, tag="bri")
        nc.sync.dma_start(bri_s[:], bri[:])
        bnin_s = const.tile([1, H], F32R, tag="bnin")
        nc.sync.dma_start(bnin_s[:], bnin[:])
        bnh_s = const.tile([1, H], F32R, tag="bnh")
        nc.sync.dma_start(bnh_s[:], bnh[:])

        whh_pool = ctx.enter_context(tc.tile_pool(name="whh_pool", bufs=1))

        a_pool = ctx.enter_context(tc.tile_pool(name="a_pool", bufs=1))
        a_t = [
            a_pool.tile([128, H3], BF16, tag=f"a_{g}", name=f"a_{g}") for g in range(8)
        ]

        # f1 stays resident through pass 2 (loaded lazily, just before use);
        # w2 (n-chunk of wihT) + whh loaded after pass-0 emission.
        f1_pool = ctx.enter_context(tc.tile_pool(name="f1_pool", bufs=1))
        f1_tiles = {}

        def f1_tile(k):
            if k not in f1_tiles:
                tt = f1_pool.tile([128, BT], F32R, tag=f"f1_{k}", name=f"f1_{k}")
                nc.sync.dma_start(tt[:], f1T[128 * k : 128 * (k + 1), :])
                f1_tiles[k] = tt
            return f1_tiles[k]

        w2_pool = ctx.enter_context(tc.tile_pool(name="w2_pool", bufs=1))

        # ---- Phase 1, passes 0/1 (r and i chunks of A, all 8 bt-tiles) ----
        with tc.tile_pool(name="f0_pool", bufs=1) as f0_pool, tc.tile_pool(
            name="wstream", bufs=3
        ) as wstream, tc.tile_pool(name="pre_psum", bufs=1, space="PSUM") as pre_psum:
            f0_tiles = {}

            def f0_tile(k):
                if k not in f0_tiles:
                    tt = f0_pool.tile([128, BT], F32R, tag=f"f0_{k}", name=f"f0_{k}")
                    nc.sync.dma_start(tt[:], f0T[128 * k : 128 * (k + 1), :])
                    f0_tiles[k] = tt
                return f0_tiles[k]
            for p in range(2):
                lo = 512 * p
                ps = [
                    pre_psum.tile([128, 512], F32, tag=f"ps{g}", name=f"ps{g}")
                    for g in range(8)
                ]
                for g in range(8):
                    nc.tensor.matmul(
                        ps[g][:], ones[:, :128], bri_s[:, lo : lo + 512],
                        start=True, stop=False,
                    )
                for k in range(K_IN):
                    wt = wstream.tile([128, 512], F32R, tag="w", name="w")
                    nc.sync.dma_start(
                        wt[:], wihT[128 * k : 128 * (k + 1), lo : lo + 512]
                    )
                    for g in range(8):
                        nc.tensor.matmul(
                            ps[g][:],
                            f1_tile(k)[:, 128 * g : 128 * (g + 1)],
                            wt[:],
                            start=False, stop=False,
                        )
                for k in range(K_FS):
                    wt = wstream.tile([128, 512], F32R, tag="w", name="w")
                    nc.sync.dma_start(
                        wt[:], wfhT[128 * k : 128 * (k + 1), lo : lo + 512]
                    )
                    for g in range(8):
                        nc.tensor.matmul(
                            ps[g][:],
                            f0_tile(k)[:, 128 * g : 128 * (g + 1)],
                            wt[:],
                            start=False, stop=(k == K_FS - 1),
                        )
                for g in range(8):
                    nc.vector.tensor_copy(a_t[g][:, lo : lo + 512], ps[g][:])

        whh_t = []
        for k in range(K_H):
            tt = whh_pool.tile([128, H3], F32R, tag=f"whh_{k}", name=f"whh_{k}")
            nc.sync.dma_start(tt[:], whhT[128 * k : 128 * (k + 1), :])
            whh_t.append(tt)
        w2_t = []
        for k in range(K_IN):
            tt = w2_pool.tile([128, 512], F32R, tag=f"w2_{k}", name=f"w2_{k}")
            nc.sync.dma_start(tt[:], wihT[128 * k : 128 * (k + 1), 1024:1536])
            w2_t.append(tt)

        # ---- Phase 1 pass 2 (n chunk) interleaved with Phase 2 recurrence ----
        h_pool = ctx.enter_context(tc.tile_pool(name="h_pool", bufs=2))
        hT_pool = ctx.enter_context(tc.tile_pool(name="hT_pool", bufs=2))
        g_pool = ctx.enter_context(tc.tile_pool(name="g_pool", bufs=2))
        at_pool = ctx.enter_context(tc.tile_pool(name="at_pool", bufs=4))
        rec_psum = ctx.enter_context(
            tc.tile_pool(name="rec_psum", bufs=2, space="PSUM")
        )
        n2_psum = ctx.enter_context(tc.tile_pool(name="n2_psum", bufs=1, space="PSUM"))

        h = h_pool.tile([BL, H], F32, tag="h")
        nc.vector.memset(h[:], 0.0)
        hT = hT_pool.tile([128, K_H * BL], F32R, tag="hT")
        nc.sync.dma_start(hT[:], zeros_d[:])

        at_tiles = {}

        def stage_at(t):
            at = at_pool.tile([BL, H3], BF16, tag="at", name="at")
            nc.sync.dma_start(
                at[:], a_t[t // 8][BL * (t % 8) : BL * (t % 8) + BL, :]
            )
            at_tiles[t] = at

        def emit_inject(t):
            """Allocate psums for step t and inject A_r, A_i, b_hh[n]."""
            psr = rec_psum.tile([BL, H], F32, tag="psR", name="psR")
            psi = rec_psum.tile([BL, H], F32, tag="psI", name="psI")
            psn = rec_psum.tile([BL, H], F32, tag="psN", name="psN")
            at = at_tiles[t]
            nc.tensor.matmul(
                psr[:], identb[:], at[:, 0:512], start=True, stop=False
            )
            nc.tensor.matmul(
                psi[:], identb[:], at[:, 512:1024], start=True, stop=False
            )
            nc.tensor.matmul(
                psn[:], ones[:, :BL], bnh_s[:], start=True, stop=False
            )
            return psr, psi, psn

        ps2_cur = {}

        def emit_pass2_half(g, half):
            """n-chunk of A for bt-tile g, split in two matmul half-groups."""
            if half == 0:
                ps2 = n2_psum.tile([128, 512], F32, tag="ps2", name="ps2")
                ps2_cur[g] = ps2
                nc.tensor.matmul(
                    ps2[:], ones[:, :128], bnin_s[:], start=True, stop=False
                )
                ks = range(0, K_IN // 2)
            else:
                ps2 = ps2_cur.pop(g)
                ks = range(K_IN // 2, K_IN)
            for k in ks:
                nc.tensor.matmul(
                    ps2[:],
                    f1_tile(k)[:, 128 * g : 128 * (g + 1)],
                    w2_t[k][:],
                    start=False, stop=(k == K_IN - 1),
                )
            if half == 1:
                nc.vector.tensor_copy(a_t[g][:, 1024:1536], ps2[:])

        # chunk order within a step: r (0), n (2), i (1)
        CH = [0, 2, 1]

        emit_pass2_half(0, 0)
        emit_pass2_half(0, 1)
        stage_at(0)
        ps_cur = emit_inject(0)

        for t in range(T):
            psr, psi, psn = ps_cur
            at = at_tiles.pop(t)
            for n in CH:
                dst = (psr, psi, psn)[n]
                for k in range(K_H):
                    nc.tensor.matmul(
                        dst[:],
                        hT[:, BL * k : BL * (k + 1)],
                        whh_t[k][:, 512 * n : 512 * (n + 1)],
                        start=False, stop=(k == K_H - 1),
                    )
            # interleave pass-2 tile g+1 / staging / next-step injects here so
            # they land in the PE queue before the transposes (which stall)
            if t % 8 == 0 and t // 8 + 1 < 8:
                emit_pass2_half(t // 8 + 1, 0)
            if t % 8 == 4 and t // 8 + 1 < 8:
                emit_pass2_half(t // 8 + 1, 1)
            if t + 1 < T:
                stage_at(t + 1)
                ps_cur = emit_inject(t + 1)

            r = g_pool.tile([BL, H], BF16, tag="r")
            nc.scalar.activation(r[:], psr[:], AF.Sigmoid)
            tmp = g_pool.tile([BL, H], BF16, tag="tmp")
            nc.vector.scalar_tensor_tensor(
                tmp[:], psn[:], 0.0, r[:], ALU.bypass, ALU.mult
            )
            tmp2 = g_pool.tile([BL, H], BF16, tag="tmp2")
            i_tmp2 = nc.vector.tensor_add(tmp2[:], tmp[:], at[:, 1024:1536])
            ri = g_pool.tile([BL, H], F32, tag="ri")
            nc.scalar.activation(ri[:], psi[:], AF.Sigmoid)
            nt = g_pool.tile([BL, H], F32, tag="nt")
            nc.scalar.activation(nt[:], tmp2[:], AF.Tanh)
            # h' = nt + ri*h - ri*nt, accumulated transposed in PSUM:
            # q1 = ri*h runs during tanh; q2 = -ri*nt is the only post-tanh op
            q1 = g_pool.tile([BL, H], F32, tag="q1")
            i_q1 = nc.vector.tensor_mul(q1[:], ri[:], h[:])
            tile.add_dep_helper(i_q1.ins, i_tmp2.ins, sync=False, reason="dve order")
            q2 = g_pool.tile([BL, H], F32, tag="q2")
            nc.vector.scalar_tensor_tensor(
                q2[:], nt[:], -1.0, ri[:], ALU.mult, ALU.mult
            )
            if t < T - 1:
                pst = n2_psum.tile([128, K_H * BL], F32, tag="pst", name="pst")
                for src_i, src in enumerate((nt, q1, q2)):
                    for j in range(K_H):
                        nc.tensor.matmul(
                            pst[:, BL * j : BL * (j + 1)],
                            src[:, 128 * j : 128 * (j + 1)],
                            ident[:],
                            is_transpose=True,
                            start=(src_i == 0 and j == 0),
                            stop=(src_i == 2 and j == K_H - 1),
                        )
                hT_new = hT_pool.tile([128, K_H * BL], F32R, tag="hT")
                for j in range(K_H):
                    i_copy = nc.vector.tensor_copy(
                        hT_new[:, BL * j : BL * (j + 1)],
                        pst[:, BL * j : BL * (j + 1)],
                    )
            w_s = g_pool.tile([BL, H], F32, tag="w_s")
            i_ws = nc.vector.tensor_add(w_s[:], nt[:], q2[:])
            h_new = h_pool.tile([BL, H], F32, tag="h")
            i_hn = nc.vector.tensor_add(h_new[:], w_s[:], q1[:])
            if t < T - 1:
                # h_new is off the critical path: keep it behind the hT copy
                tile.add_dep_helper(i_ws.ins, i_copy.ins, sync=False, reason="order")
            nc.sync.dma_start(eo[:, t, :], h_new[:])
            if t < T - 1:
                h, hT = h_new, hT_new

    nc.compile()
    return nc


def _prep_inputs(feat0, feat1, W_ih, b_ih, W_hh, b_hh, W_fh, b_fh):
    import ml_dtypes

    wihT = np.ascontiguousarray(W_ih.T)
    wfhT = np.ascontiguousarray(W_fh.T)
    whhT = np.ascontiguousarray(W_hh.T)
    bri = np.ascontiguousarray(
        (b_ih[: 2 * H] + b_fh + b_hh[: 2 * H]).reshape(1, 2 * H)
    )
    bnin = np.ascontiguousarray(b_ih[2 * H :].reshape(1, H))
    bnh = np.ascontiguousarray(b_hh[2 * H :].reshape(1, H))
    identb = np.eye(BL).astype(ml_dtypes.bfloat16)
    in_maps = []
    for c in range(NC):
        sl = slice(BL * c, BL * (c + 1))
        # column order bt' = t*BL + b so step t reads 16 contiguous partitions
        f1c = np.ascontiguousarray(feat1[sl].transpose(2, 1, 0).reshape(IN, BT))
        f0c = np.ascontiguousarray(feat0[sl].transpose(2, 1, 0).reshape(FS, BT))
        in_maps.append(
            {
                "f1T": f1c,
                "f0T": f0c,
                "wihT": wihT,
                "wfhT": wfhT,
                "whhT": whhT,
                "bri": bri,
                "bnin": bnin,
                "bnh": bnh,
                "ones_d": np.ones((1, 128), np.float32),
                "zeros_d": np.zeros((128, 64), np.float32),
                "identb_d": identb,
            }
        )
    return in_maps


def kernel(feat0, feat1, W_ih, b_ih, W_hh, b_hh, W_fh, b_fh):
    feat0 = np.asarray(feat0, np.float32)
    feat1 = np.asarray(feat1, np.float32)
    W_ih = np.asarray(W_ih, np.float32)
    b_ih = np.asarray(b_ih, np.float32)
    W_hh = np.asarray(W_hh, np.float32)
    b_hh = np.asarray(b_hh, np.float32)
    W_fh = np.asarray(W_fh, np.float32)
    b_fh = np.asarray(b_fh, np.float32)

    if "nc" not in _cache:
        _cache["nc"] = _build_program()
    nc = _cache["nc"]

    in_maps = _prep_inputs(feat0, feat1, W_ih, b_ih, W_hh, b_hh, W_fh, b_fh)
    res = run_bass_kernel_spmd(nc, in_maps, core_ids=list(range(NC)))
    eo = np.empty((B, T, H), np.float32)
    for c in range(NC):
        eo[BL * c : BL * (c + 1)] = res.results[c]["eo"]
    hT = np.ascontiguousarray(eo[:, -1, :])
    return eo, hT


# revision 42
# speedup vs baseline: 9438.0760x; 65.6772x over previous
"""Trainium2 Bass kernel for Encoder_Baseline (GRU_with_GCC over stream 1).

Distribution: data-parallel over batch (B=128 -> 16 rows per core, 8 cores,
one SPMD NEFF).  Each core returns its eo shard; the host concatenates and
derives hT = eo[:, -1].

Per-core program (single TileContext, engines overlapped):

Phase 1 computes the time-parallel gate pre-activations
    A[bt, 0:512]    = feat1 @ W_ih.T[:,0:512]   + feat0 @ W_fh.T[:,0:512]  + b_r
    A[bt, 512:1024] = feat1 @ W_ih.T[:,512:1024]+ feat0 @ W_fh.T[:,512:]   + b_i
    A[bt, 1024:]    = feat1 @ W_ih.T[:,1024:]   + b_ih[1024:]
with fp32r matmuls (full PE rate at N=512), biases injected via K=1
ones-matmuls, results stored bf16 in SBUF.  Only the r-chunk pass and one
tile-pair of the i-chunk run up front; the rest of the i-chunk and all of the
n-chunk are emitted as small work items interleaved into the recurrence so
their matmuls fill PE idle gaps during the per-step gate chains.

Phase 2 runs the 64 sequential GRU steps batch-major ([16, *] tiles):
  - h.T (hidden-major, f32r) is the matmul stationary operand; the moving
    operand is W_hh.T, streamed 512 columns per matmul (3 gate chunks x 4
    K-tiles, separate PSUM tiles per chunk so sigmoid(r) starts after only
    4 matmuls).
  - A_t slices are staged to partition 0 by SBUF->SBUF DMA and injected into
    PSUM by small identity matmuls emitted one step ahead (off the critical
    path); b_hh[n] is injected the same way, so the n-gate PSUM holds
    b_hh-biased h_n directly.
  - gates: r = sigmoid(psR); tmp = r * psN (fused scalar_tensor_tensor);
    n = tanh(tmp + A_n); i = sigmoid(psI).
  - h' = n + i*h - i*n is never materialized on the critical path: q1 = i*h
    (runs during tanh) and q2 = -i*n are transposed by the PE and summed with
    n.T directly in PSUM via matmul has_written accumulation, giving the next
    step's stationary h'.T after a single PSUM->SBUF copy.  The batch-major
    h' (for the eo DMA and the next q1) is assembled off-path.
"""

import sys

for _p in ("/opt/trn_rl_repo",):
    if _p not in sys.path:
        sys.path.insert(0, _p)

from contextlib import ExitStack

import numpy as np

import concourse.bass as bass
import concourse.mybir as mybir
import concourse.tile as tile
from concourse import bacc
from concourse.bass_utils import run_bass_kernel_spmd
from concourse.masks import make_identity

F32 = mybir.dt.float32
F32R = mybir.dt.float32r
BF16 = mybir.dt.bfloat16
AF = mybir.ActivationFunctionType
ALU = mybir.AluOpType

B, T, H, IN, FS = 128, 64, 512, 2048, 1536
NC = 8
BL = B // NC          # 16 batch rows per core
BT = BL * T           # 1024 rows per core
H3 = 3 * H            # 1536
K_IN = IN // 128      # 16
K_FS = FS // 128      # 12
K_H = H // 128        # 4

_cache = {}


def _build_program():
    nc = bacc.Bacc("TRN2", target_bir_lowering=False, debug=False)

    f1T = nc.dram_tensor("f1T", [IN, BT], F32R, kind="ExternalInput").ap()
    f0T = nc.dram_tensor("f0T", [FS, BT], F32R, kind="ExternalInput").ap()
    wihT = nc.dram_tensor("wihT", [IN, H3], F32R, kind="ExternalInput").ap()
    wfhT = nc.dram_tensor("wfhT", [FS, 2 * H], F32R, kind="ExternalInput").ap()
    whhT = nc.dram_tensor("whhT", [H, H3], F32R, kind="ExternalInput").ap()
    bri = nc.dram_tensor("bri", [1, 2 * H], F32R, kind="ExternalInput").ap()
    bnin = nc.dram_tensor("bnin", [1, H], F32R, kind="ExternalInput").ap()
    bnh = nc.dram_tensor("bnh", [1, H], F32R, kind="ExternalInput").ap()
    ones_d = nc.dram_tensor("ones_d", [1, 128], F32R, kind="ExternalInput").ap()
    zeros_d = nc.dram_tensor("zeros_d", [128, 64], F32R, kind="ExternalInput").ap()
    identb_d = nc.dram_tensor("identb_d", [BL, BL], BF16, kind="ExternalInput").ap()
    eo = nc.dram_tensor("eo", [BL, T, H], F32, kind="ExternalOutput").ap()

    with tile.TileContext(nc) as tc, ExitStack() as ctx:
        const = ctx.enter_context(tc.tile_pool(name="const", bufs=1))
        ident = const.tile([BL, BL], F32, tag="ident")
        make_identity(nc, ident[:])
        identb = const.tile([BL, BL], BF16, tag="identb")
        nc.sync.dma_start(identb[:], identb_d[:])
        ones = const.tile([1, 128], F32R, tag="ones")
        nc.sync.dma_start(ones[:], ones_d[:])
        bri_s = const.tile([1, 2 * H], F32R, tag="bri")
        nc.sync.dma_start(bri_s[:], bri[:])
        bnin_s = const.tile([1, H], F32R, tag="bnin")
        nc.sync.dma_start(bnin_s[:], bnin[:])
        bnh_s = const.tile([1, H], F32R, tag="bnh")
        nc.sync.dma_start(bnh_s[:], bnh[:])

        whh_pool = ctx.enter_context(tc.tile_pool(name="whh_pool", bufs=1))

        a_pool = ctx.enter_context(tc.tile_pool(name="a_pool", bufs=1))
        a_t = [
            a_pool.tile([128, H3], BF16, tag=f"a_{g}", name=f"a_{g}") for g in range(8)
        ]

        # f1 stays resident through pass 2 (loaded lazily, just before use);
        # w2 (n-chunk of wihT) + whh loaded after pass-0 emission.
        f1_pool = ctx.enter_context(tc.tile_pool(name="f1_pool", bufs=1))
        f1_tiles = {}

        def f1_tile(k):
            if k not in f1_tiles:
                tt = f1_pool.tile([128, BT], F32R, tag=f"f1_{k}", name=f"f1_{k}")
                nc.sync.dma_start(tt[:, 0:512], f1T[128 * k : 128 * (k + 1), 0:512])
                nc.sync.dma_start(
                    tt[:, 512:BT], f1T[128 * k : 128 * (k + 1), 512:BT]
                )
                f1_tiles[k] = tt
            return f1_tiles[k]

        w2_pool = ctx.enter_context(tc.tile_pool(name="w2_pool", bufs=1))

        wstream = ctx.enter_context(tc.tile_pool(name="wstream", bufs=5))

        # ---- Phase 1 pass 0 (r chunk of A, all 8 bt-tiles) ----
        with tc.tile_pool(name="pre_psum", bufs=1, space="PSUM") as pre_psum:
            ps = [
                pre_psum.tile([128, 512], F32, tag=f"ps{g}", name=f"ps{g}")
                for g in range(8)
            ]
            for g in range(8):
                nc.tensor.matmul(
                    ps[g][:], ones[:, :128], bri_s[:, 0:512],
                    start=True, stop=False,
                )
            for k in range(K_IN):
                wt = wstream.tile([128, 512], F32R, tag="w", name="w")
                nc.sync.dma_start(wt[:], wihT[128 * k : 128 * (k + 1), 0:512])
                for g in range(8):
                    nc.tensor.matmul(
                        ps[g][:],
                        f1_tile(k)[:, 128 * g : 128 * (g + 1)],
                        wt[:],
                        start=False, stop=False,
                    )
            for k in range(K_FS):
                wt = wstream.tile([128, 512], F32R, tag="w", name="w")
                nc.sync.dma_start(wt[:], wfhT[128 * k : 128 * (k + 1), 0:512])
                f0w = wstream.tile([128, BT], F32R, tag="f0w", name="f0w", bufs=2)
                nc.sync.dma_start(f0w[:, 0:512], f0T[128 * k : 128 * (k + 1), 0:512])
                nc.sync.dma_start(
                    f0w[:, 512:BT], f0T[128 * k : 128 * (k + 1), 512:BT]
                )
                for g in range(8):
                    nc.tensor.matmul(
                        ps[g][:],
                        f0w[:, 128 * g : 128 * (g + 1)],
                        wt[:],
                        start=False, stop=(k == K_FS - 1),
                    )
            for g in range(8):
                nc.vector.tensor_copy(a_t[g][:, 0:512], ps[g][:])

        whh_t = []
        for k in range(K_H):
            tt = whh_pool.tile([128, H3], F32R, tag=f"whh_{k}", name=f"whh_{k}")
            nc.sync.dma_start(tt[:], whhT[128 * k : 128 * (k + 1), :])
            whh_t.append(tt)
        w2_t = []
        for k in range(K_IN):
            tt = w2_pool.tile([128, 512], F32R, tag=f"w2_{k}", name=f"w2_{k}")
            nc.sync.dma_start(tt[:], wihT[128 * k : 128 * (k + 1), 1024:1536])
            w2_t.append(tt)

        # ---- Phase 1 pass 2 (n chunk) interleaved with Phase 2 recurrence ----
        h_pool = ctx.enter_context(tc.tile_pool(name="h_pool", bufs=2))
        hT_pool = ctx.enter_context(tc.tile_pool(name="hT_pool", bufs=2))
        g_pool = ctx.enter_context(tc.tile_pool(name="g_pool", bufs=2))
        at_pool = ctx.enter_context(tc.tile_pool(name="at_pool", bufs=2))
        rec_psum = ctx.enter_context(
            tc.tile_pool(name="rec_psum", bufs=1, space="PSUM")
        )
        n2_psum = ctx.enter_context(tc.tile_pool(name="n2_psum", bufs=1, space="PSUM"))

        # ---- Phase 1 passes 0 and 1 (r and i chunks of A) as a work queue
        # of small PE items drip-fed into the recurrence's PE idle gaps
        # (W and f0 streamed per tile-pair; f1 read from resident tiles) ----
        ps1_cur = {}

        def p1_inject(c, gp):
            lo = 512 * c

            def _f():
                for gi in range(2):
                    g = 2 * gp + gi
                    p = n2_psum.tile([128, 512], F32, tag="ps1", name="ps1", bufs=2)
                    ps1_cur[g] = p
                    nc.tensor.matmul(
                        p[:], ones[:, :128], bri_s[:, lo : lo + 512],
                        start=True, stop=False,
                    )
            return _f

        def p1_wih(c, gp, k):
            lo = 512 * c

            def _f():
                wt = wstream.tile([128, 512], F32R, tag="w", name="w")
                nc.sync.dma_start(wt[:], wihT[128 * k : 128 * (k + 1), lo : lo + 512])
                for gi in range(2):
                    g = 2 * gp + gi
                    nc.tensor.matmul(
                        ps1_cur[g][:],
                        f1_tile(k)[:, 128 * g : 128 * (g + 1)],
                        wt[:],
                        start=False, stop=False,
                    )
            return _f

        def p1_wfh(c, gp, k):
            lo = 512 * c

            def _f():
                wt = wstream.tile([128, 512], F32R, tag="w", name="w")
                nc.sync.dma_start(wt[:], wfhT[128 * k : 128 * (k + 1), lo : lo + 512])
                f0s = wstream.tile([128, 256], F32R, tag="f0s", name="f0s", bufs=3)
                nc.sync.dma_start(
                    f0s[:], f0T[128 * k : 128 * (k + 1), 256 * gp : 256 * (gp + 1)]
                )
                for gi in range(2):
                    g = 2 * gp + gi
                    nc.tensor.matmul(
                        ps1_cur[g][:],
                        f0s[:, 128 * gi : 128 * (gi + 1)],
                        wt[:],
                        start=False, stop=(k == K_FS - 1),
                    )
            return _f

        def p1_evac(c, gp):
            lo = 512 * c

            def _f():
                for gi in range(2):
                    g = 2 * gp + gi
                    nc.vector.tensor_copy(
                        a_t[g][:, lo : lo + 512], ps1_cur.pop(g)[:]
                    )
            return _f

        def p1_pair_items(c, gp):
            items = [p1_inject(c, gp)]
            items += [p1_wih(c, gp, k) for k in range(K_IN)]
            items += [p1_wfh(c, gp, k) for k in range(K_FS)]
            items.append(p1_evac(c, gp))
            return items

        # i-chunk pair 0 emitted before the recurrence starts
        for it in p1_pair_items(1, 0):
            it()
        p1_queue = []
        for gp in range(1, 4):
            p1_queue.extend(p1_pair_items(1, gp))

        h = h_pool.tile([BL, H], F32, tag="h")
        nc.vector.memset(h[:], 0.0)
        hT = hT_pool.tile([128, K_H * BL], F32R, tag="hT")
        nc.sync.dma_start(hT[:], zeros_d[:])

        at_tiles = {}

        def stage_at(t):
            at = at_pool.tile([BL, H3], BF16, tag="at", name="at")
            nc.sync.dma_start(
                at[:], a_t[t // 8][BL * (t % 8) : BL * (t % 8) + BL, :]
            )
            at_tiles[t] = at

        def emit_inject(t):
            """Allocate psums for step t and inject A_r, A_i, b_hh[n]."""
            psr = rec_psum.tile([BL, H], F32, tag="psR", name="psR")
            psi = rec_psum.tile([BL, H], F32, tag="psI", name="psI")
            psn = rec_psum.tile([BL, H], F32, tag="psN", name="psN")
            at = at_tiles[t]
            nc.tensor.matmul(
                psr[:], identb[:], at[:, 0:512], start=True, stop=False
            )
            nc.tensor.matmul(
                psi[:], identb[:], at[:, 512:1024], start=True, stop=False
            )
            nc.tensor.matmul(
                psn[:], ones[:, :BL], bnh_s[:], start=True, stop=False
            )
            return psr, psi, psn

        ps2_cur = {}

        def emit_pass2_half(g, half):
            """n-chunk of A for bt-tile g, split in two matmul half-groups."""
            if half == 0:
                ps2 = n2_psum.tile([128, 512], F32, tag="ps2", name="ps2")
                ps2_cur[g] = ps2
                nc.tensor.matmul(
                    ps2[:], ones[:, :128], bnin_s[:], start=True, stop=False
                )
                ks = range(0, K_IN // 2)
            else:
                ps2 = ps2_cur.pop(g)
                ks = range(K_IN // 2, K_IN)
            for k in ks:
                nc.tensor.matmul(
                    ps2[:],
                    f1_tile(k)[:, 128 * g : 128 * (g + 1)],
                    w2_t[k][:],
                    start=False, stop=(k == K_IN - 1),
                )
            if half == 1:
                nc.vector.tensor_copy(a_t[g][:, 1024:1536], ps2[:])

        # chunk order within a step: r (0), n (2), i (1)
        CH = [0, 2, 1]

        emit_pass2_half(0, 0)
        emit_pass2_half(0, 1)
        stage_at(0)
        ps_cur = emit_inject(0)

        for t in range(T):
            psr, psi, psn = ps_cur
            at = at_tiles.pop(t)
            for n in CH:
                dst = (psr, psi, psn)[n]
                for k in range(K_H):
                    nc.tensor.matmul(
                        dst[:],
                        hT[:, BL * k : BL * (k + 1)],
                        whh_t[k][:, 512 * n : 512 * (n + 1)],
                        start=False, stop=(k == K_H - 1),
                    )
            # interleave pass-2 tile g+1 / staging / next-step injects here so
            # they land in the PE queue before the transposes (which stall)
            if t % 8 == 0 and t // 8 + 1 < 8:
                emit_pass2_half(t // 8 + 1, 0)
            if t % 8 == 4 and t // 8 + 1 < 8:
                emit_pass2_half(t // 8 + 1, 1)
            # drip pass-1 items into PE idle gaps
            for _ in range(3):
                if p1_queue:
                    p1_queue.pop(0)()
            if t + 1 < T:
                stage_at(t + 1)
                ps_cur = emit_inject(t + 1)

            r = g_pool.tile([BL, H], BF16, tag="r")
            nc.scalar.activation(r[:], psr[:], AF.Sigmoid)
            tmp = g_pool.tile([BL, H], BF16, tag="tmp")
            nc.vector.scalar_tensor_tensor(
                tmp[:], psn[:], 0.0, r[:], ALU.bypass, ALU.mult
            )
            tmp2 = g_pool.tile([BL, H], BF16, tag="tmp2")
            i_tmp2 = nc.vector.tensor_add(tmp2[:], tmp[:], at[:, 1024:1536])
            ri = g_pool.tile([BL, H], F32, tag="ri")
            nc.scalar.activation(ri[:], psi[:], AF.Sigmoid)
            nt = g_pool.tile([BL, H], F32, tag="nt")
            nc.scalar.activation(nt[:], tmp2[:], AF.Tanh)
            # h' = nt + ri*h - ri*nt, accumulated transposed in PSUM:
            # q1 = ri*h runs during tanh; q2 = -ri*nt is the only post-tanh op
            q1 = g_pool.tile([BL, H], F32, tag="q1")
            i_q1 = nc.vector.tensor_mul(q1[:], ri[:], h[:])
            tile.add_dep_helper(i_q1.ins, i_tmp2.ins, sync=False, reason="dve order")
            q2 = g_pool.tile([BL, H], F32, tag="q2")
            nc.vector.scalar_tensor_tensor(
                q2[:], nt[:], -1.0, ri[:], ALU.mult, ALU.mult
            )
            if t < T - 1:
                pst = n2_psum.tile([128, K_H * BL], F32, tag="pst", name="pst")
                for src_i, src in enumerate((nt, q1, q2)):
                    for j in range(K_H):
                        nc.tensor.matmul(
                            pst[:, BL * j : BL * (j + 1)],
                            src[:, 128 * j : 128 * (j + 1)],
                            ident[:],
                            is_transpose=True,
                            start=(src_i == 0 and j == 0),
                            stop=(src_i == 2 and j == K_H - 1),
                        )
                hT_new = hT_pool.tile([128, K_H * BL], F32R, tag="hT")
                i_copy = nc.vector.tensor_copy(hT_new[:], pst[:])
            i_ws = nc.vector.tensor_add(q2[:], nt[:], q2[:])
            h_new = h_pool.tile([BL, H], F32, tag="h")
            i_hn = nc.vector.tensor_add(h_new[:], q2[:], q1[:])
            if t < T - 1:
                # h_new is off the critical path: keep it behind the hT copy
                tile.add_dep_helper(i_ws.ins, i_copy.ins, sync=False, reason="order")
            nc.sync.dma_start(eo[:, t, :], h_new[:])
            if t < T - 1:
                h, hT = h_new, hT_new

    nc.compile()
    return nc


def _prep_inputs(feat0, feat1, W_ih, b_ih, W_hh, b_hh, W_fh, b_fh):
    import ml_dtypes

    wihT = np.ascontiguousarray(W_ih.T)
    wfhT = np.ascontiguousarray(W_fh.T)
    whhT = np.ascontiguousarray(W_hh.T)
    bri = np.ascontiguousarray(
        (b_ih[: 2 * H] + b_fh + b_hh[: 2 * H]).reshape(1, 2 * H)
    )
    bnin = np.ascontiguousarray(b_ih[2 * H :].reshape(1, H))
    bnh = np.ascontiguousarray(b_hh[2 * H :].reshape(1, H))
    identb = np.eye(BL).astype(ml_dtypes.bfloat16)
    in_maps = []
    for c in range(NC):
        sl = slice(BL * c, BL * (c + 1))
        # column order bt' = t*BL + b so step t reads 16 contiguous partitions
        f1c = np.ascontiguousarray(feat1[sl].transpose(2, 1, 0).reshape(IN, BT))
        f0c = np.ascontiguousarray(feat0[sl].transpose(2, 1, 0).reshape(FS, BT))
        in_maps.append(
            {
                "f1T": f1c,
                "f0T": f0c,
                "wihT": wihT,
                "wfhT": wfhT,
                "whhT": whhT,
                "bri": bri,
                "bnin": bnin,
                "bnh": bnh,
                "ones_d": np.ones((1, 128), np.float32),
                "zeros_d": np.zeros((128, 64), np.float32),
                "identb_d": identb,
            }
        )
    return in_maps


def kernel(feat0, feat1, W_ih, b_ih, W_hh, b_hh, W_fh, b_fh):
    feat0 = np.asarray(feat0, np.float32)
    feat1 = np.asarray(feat1, np.float32)
    W_ih = np.asarray(W_ih, np.float32)
    b_ih = np.asarray(b_ih, np.float32)
    W_hh = np.asarray(W_hh, np.float32)
    b_hh = np.asarray(b_hh, np.float32)
    W_fh = np.asarray(W_fh, np.float32)
    b_fh = np.asarray(b_fh, np.float32)

    if "nc" not in _cache:
        _cache["nc"] = _build_program()
    nc = _cache["nc"]

    in_maps = _prep_inputs(feat0, feat1, W_ih, b_ih, W_hh, b_hh, W_fh, b_fh)
    res = run_bass_kernel_spmd(nc, in_maps, core_ids=list(range(NC)))
    eo = np.empty((B, T, H), np.float32)
    for c in range(NC):
        eo[BL * c : BL * (c + 1)] = res.results[c]["eo"]
    hT = np.ascontiguousarray(eo[:, -1, :])
    return eo, hT
